# revision 1
# baseline (speedup 1.0000x reference)
"""InternLM3 self-attention (prefill, GQA, RoPE) on 8 Trainium2 cores.

Tensor-parallel over heads: core r owns q heads 4r..4r+3 and kv head r
(wqkv column shards, wo row shards).  Each core computes its partial
output projection; the 8 partials are summed on the host (an on-device
all-reduce of 32 MB runs at ~32 GB/s through ncfw and would dominate the
kernel, so the reduction is done host-side).

Matmuls run in float32r (TF32-like fast fp32 mode, 1 cycle/row at
N>=512 vs 4 for plain fp32) with fp32 PSUM accumulation.

Device-side layout trick: everything is computed transposed
(qkv^T = wqkv_shard^T @ hidden^T) so that
  - wqkv loads land directly as the stationary operand,
  - q^T/k^T slices feed the scores matmul with head_dim on partitions,
  - scores come out as S^T [k, q], so exp(S^T) feeds the PV matmul
    directly (contraction over k on partitions) with zero transposes,
  - attn^T slices are exactly the stationary operand of the wo matmul.
The only transposes are hidden^T (done host-side, it is an input-layout
choice) and v^T -> v (16 tiny PE transposes).
"""

import numpy as np

import concourse.bass as bass
import concourse.bacc as bacc
import concourse.mybir as mybir
import concourse.tile as tile
from concourse.bass_utils import run_bass_kernel_spmd

T = 2048
H = 4096
NH = 32
NKV = 8
HD = 128
HALF = HD // 2
BASE = 1000000.0
NCORES = 8
QH = NH // NCORES            # 4 q heads per core
QCOLS = QH * HD              # 512
SH_COLS = QCOLS + 2 * HD     # 768 wqkv cols per core
NEG = -1e30

P = 128
TC = 512                     # token chunk (matmul moving dim)
NT = T // TC                 # 4
NHC = H // P                 # 32 contraction chunks for qkv
NQC = SH_COLS // P           # 6 qkv col chunks
NKC = T // P                 # 16 k chunks
NOC = H // TC                # 8 output col chunks
NTC16 = T // P               # 16 token chunks of 128

f32 = mybir.dt.float32
f32r = mybir.dt.float32r

_COMPILED = None


def _build():
    nc = bacc.Bacc("TRN2", target_bir_lowering=False, debug=False,
                   num_devices=NCORES)

    hidT = nc.dram_tensor("hidT", [H, T], f32r, kind="ExternalInput").ap()
    wqkv_s = nc.dram_tensor("wqkv_s", [H, SH_COLS], f32r,
                            kind="ExternalInput").ap()
    wo_s = nc.dram_tensor("wo_s", [QCOLS, H], f32r,
                          kind="ExternalInput").ap()
    cosq = nc.dram_tensor("cosq", [P, T], f32, kind="ExternalInput").ap()
    sinq = nc.dram_tensor("sinq", [P, T], f32, kind="ExternalInput").ap()
    cosk = nc.dram_tensor("cosk", [P, T], f32, kind="ExternalInput").ap()
    sink = nc.dram_tensor("sink", [P, T], f32, kind="ExternalInput").ap()
    masks = nc.dram_tensor("masks", [P, 4, TC], f32,
                           kind="ExternalInput").ap()
    rperm = nc.dram_tensor("rperm", [P, P], f32r, kind="ExternalInput").ap()
    ident = nc.dram_tensor("ident", [P, P], f32r, kind="ExternalInput").ap()
    ones_k = nc.dram_tensor("ones_k", [P, 1], f32r,
                            kind="ExternalInput").ap()
    ones_m = nc.dram_tensor("ones_m", [1, P], f32r,
                            kind="ExternalInput").ap()
    part = nc.dram_tensor("part", [T, H], f32, kind="ExternalOutput").ap()

    with tile.TileContext(nc) as tc:
        with tc.tile_pool(name="keep", bufs=1) as keep:
            # long-lived SBUF: qkv^T [128, 6, 2048] f32r (48 KB/part)
            qkvT = keep.tile([P, NQC, T], f32r)

            # constants first: tiny DMAs, land before the bulk loads
            ct = keep.tile([P, T], f32, tag="cosq_t")
            st = keep.tile([P, T], f32, tag="sinq_t")
            ctk = keep.tile([P, T], f32, tag="cosk_t")
            stk = keep.tile([P, T], f32, tag="sink_t")
            mt = keep.tile([P, 4, TC], f32, tag="masks_t")
            rp = keep.tile([P, P], f32r, tag="rperm_t")
            idt = keep.tile([P, P], f32r, tag="ident_t")
            o_k = keep.tile([P, 1], f32r, tag="ones_k_t")
            o_m = keep.tile([1, P], f32r, tag="ones_m_t")

            # ---------------- phase 1: qkv^T = wqkv^T @ hidden^T -------
            with tc.tile_pool(name="wq", bufs=1) as wqp, \
                 tc.tile_pool(name="hstream", bufs=4) as hsp, \
                 tc.tile_pool(name="qps", bufs=1, space="PSUM") as qpsp:
                wq = wqp.tile([P, NHC, SH_COLS], f32r)
                for h in range(NHC):
                    nc.sync.dma_start(
                        wq[:, h, :], wqkv_s[h * P:(h + 1) * P, :])
                nc.sync.dma_start(ct[:], cosq[:])
                nc.sync.dma_start(st[:], sinq[:])
                nc.sync.dma_start(ctk[:], cosk[:])
                nc.sync.dma_start(stk[:], sink[:])
                nc.sync.dma_start(mt[:], masks[:])
                nc.sync.dma_start(rp[:], rperm[:])
                nc.sync.dma_start(idt[:], ident[:])
                nc.sync.dma_start(o_k[:], ones_k[:])
                nc.sync.dma_start(o_m[:], ones_m[:])
                for t in range(NT):
                    qps = [qpsp.tile([P, TC], f32, tag=f"qps{c}",
                                     name=f"qps{c}_{t}")
                           for c in range(NQC)]
                    for h in range(NHC):
                        ht = hsp.tile([P, TC], f32r, tag="ht")
                        nc.scalar.dma_start(
                            ht[:], hidT[h * P:(h + 1) * P,
                                        t * TC:(t + 1) * TC])
                        for c in range(NQC):
                            nc.tensor.matmul(
                                qps[c][:], wq[:, h, c * P:(c + 1) * P],
                                ht[:], start=(h == 0), stop=(h == NHC - 1))
                    for c in range(NQC):
                        nc.scalar.copy(
                            qkvT[:, c, t * TC:(t + 1) * TC], qps[c][:])

            with tc.tile_pool(name="keep2", bufs=1) as keep2:
                    # ---------------- phase 3: v_nat = v^T transposed ----------
                vnat = keep2.tile([P, NKC, P], f32r, tag="vnat")
                with tc.tile_pool(name="vt_ps", bufs=4, space="PSUM") as vps:
                    for kc in range(NKC):
                        tp = vps.tile([P, P], f32r, tag="vtp")
                        nc.tensor.transpose(
                            tp[:], qkvT[:, 5, kc * P:(kc + 1) * P], idt[:])
                        nc.scalar.copy(vnat[:, kc, :], tp[:])

                # ---------------- phase 2: RoPE on q (scaled) and k --------
                with tc.tile_pool(name="rope_sb", bufs=4) as rsb, \
                     tc.tile_pool(name="rope_ps", bufs=4, space="PSUM") as rps:
                    for idx in range(QH + 1):        # 4 q heads + 1 k head
                        cos_t, sin_t = (ct, st) if idx < QH else (ctk, stk)
                        for t in range(NT):
                            sl = slice(t * TC, (t + 1) * TC)
                            x = qkvT[:, idx, sl]
                            rot = rps.tile([P, TC], f32, tag="rot")
                            nc.tensor.matmul(rot[:], rp[:], x,
                                             start=True, stop=True)
                            tmp = rsb.tile([P, TC], f32, tag="rtmp")
                            nc.vector.tensor_tensor(
                                tmp[:], rot[:], sin_t[:, sl],
                                mybir.AluOpType.mult)
                            nc.vector.tensor_tensor(
                                x, x.bitcast(f32), cos_t[:, sl],
                                mybir.AluOpType.mult)
                            nc.vector.tensor_tensor(
                                x, x.bitcast(f32), tmp[:],
                                mybir.AluOpType.add)

                # ---------------- phase 4: causal attention ----------------
                attnT = keep2.tile([P, QH, T], f32r, tag="attnT")
                with tc.tile_pool(name="att_sb", bufs=8) as asb, \
                     tc.tile_pool(name="att_sm", bufs=4) as asm_p, \
                     tc.tile_pool(name="st_ps", bufs=3, space="PSUM") as stp, \
                     tc.tile_pool(name="pv_ps", bufs=2, space="PSUM") as pvp, \
                     tc.tile_pool(name="d_ps", bufs=2, space="PSUM") as dpp, \
                     tc.tile_pool(name="rb_ps", bufs=1, space="PSUM") as rbp:
                    for head in range(QH):
                        for g in range(NT):
                            kmax = (NT // 1) * (g + 1)   # 4*(g+1) k chunks
                            qsl = slice(g * TC, (g + 1) * TC)
                            d_ps = dpp.tile([1, TC], f32, tag="d")
                            pv = pvp.tile([P, TC], f32, tag="pv")
                            es = asb.tile([P, TC], f32r, tag="esum")
                            e_prev = None
                            for kc in range(kmax):
                                st_ps = stp.tile([P, TC], f32, tag="st")
                                nc.tensor.matmul(
                                    st_ps[:],
                                    qkvT[:, QH, kc * P:(kc + 1) * P],
                                    qkvT[:, head, qsl],
                                    start=True, stop=True)
                                j = kc - 4 * g
                                if j >= 0:
                                    nc.vector.tensor_tensor(
                                        st_ps[:], st_ps[:], mt[:, j, :],
                                        mybir.AluOpType.add)
                                e = asb.tile([P, TC], f32r, tag="E",
                                             name=f"e_{head}_{g}_{kc}")
                                nc.scalar.activation(
                                    e[:], st_ps[:],
                                    mybir.ActivationFunctionType.Exp)
                                # denominator partials on DVE (frees PE)
                                if kc == 1:
                                    nc.vector.tensor_tensor(
                                        es[:], e_prev[:], e[:],
                                        mybir.AluOpType.add)
                                elif kc > 1:
                                    nc.vector.tensor_tensor(
                                        es[:], es[:], e[:],
                                        mybir.AluOpType.add)
                                e_prev = e
                                nc.tensor.matmul(
                                    pv[:], vnat[:, kc, :], e[:],
                                    start=(kc == 0), stop=(kc == kmax - 1))
                            nc.tensor.matmul(d_ps[:], o_k[:], es[:],
                                             start=True, stop=True)
                            rd = asm_p.tile([1, TC], f32, tag="rd")
                            nc.vector.reciprocal(rd[:], d_ps[:])
                            rdr = asm_p.tile([1, TC], f32r, tag="rdr")
                            nc.scalar.copy(rdr[:], rd[:])
                            rb = rbp.tile([P, TC], f32, tag="rb")
                            nc.tensor.matmul(rb[:], o_m[:], rdr[:],
                                             start=True, stop=True)
                            rbs = asm_p.tile([P, TC], f32, tag="rbs")
                            nc.scalar.copy(rbs[:], rb[:])
                            nc.vector.tensor_tensor(
                                attnT[:, head, qsl], pv[:], rbs[:],
                                mybir.AluOpType.mult)

                # ---------------- phase 5: out = attn @ wo_shard -----------
                with tc.tile_pool(name="wo_sb", bufs=3) as wsb, \
                     tc.tile_pool(name="o_sb", bufs=4) as osb, \
                     tc.tile_pool(name="o_ps", bufs=4, space="PSUM") as ops:
                    for oc in range(NOC):
                        wot = wsb.tile([P, QH, TC], f32r, tag="wot")
                        nc.sync.dma_start(
                            wot[:],
                            wo_s[:, oc * TC:(oc + 1) * TC].rearrange(
                                "(hc p) n -> p hc n", p=P))
                        for tcn in range(NTC16):
                            o_ps = ops.tile([P, TC], f32, tag="o")
                            for hc in range(QH):
                                nc.tensor.matmul(
                                    o_ps[:],
                                    attnT[:, hc, tcn * P:(tcn + 1) * P],
                                    wot[:, hc, :],
                                    start=(hc == 0), stop=(hc == QH - 1))
                            ob = osb.tile([P, TC], f32, tag="ob")
                            nc.scalar.copy(ob[:], o_ps[:])
                            nc.gpsimd.dma_start(
                                part[tcn * P:(tcn + 1) * P,
                                     oc * TC:(oc + 1) * TC], ob[:])

    nc.compile()
    return nc


def _rope_tables(positions):
    pos = positions.astype(np.float64)
    inv_freq = 1.0 / (BASE ** (np.arange(HALF, dtype=np.float64) / HALF))
    freqs = pos[:, None] * inv_freq[None, :]          # [T, 64]
    cos = np.cos(freqs)
    sin = np.sin(freqs)
    cosT = np.concatenate([cos, cos], axis=1).T       # [128, T]
    sinT = np.concatenate([-sin, sin], axis=1).T      # sign folded
    return cosT.astype(np.float32), sinT.astype(np.float32)


def kernel(positions, hidden_states, wqkv, wo):
    global _COMPILED
    if _COMPILED is None:
        _COMPILED = _build()
    nc = _COMPILED

    scale = HD ** -0.5
    cosT, sinT = _rope_tables(positions)
    cosq = np.ascontiguousarray(cosT * scale)
    sinq = np.ascontiguousarray(sinT * scale)

    hidT = np.ascontiguousarray(hidden_states.T)

    # causal mask add-tiles for the diagonal blocks, ST layout [k, q]:
    # block j (k chunk 4g+j vs q group g): valid iff 128*j + kl <= ql
    kl = np.arange(P)[:, None]
    ql = np.arange(TC)[None, :]
    masks = np.stack(
        [np.where(P * j + kl <= ql, 0.0, NEG) for j in range(4)],
        axis=1).astype(np.float32)                    # [128, 4, 512]

    rperm = np.zeros((P, P), dtype=np.float32)
    for m in range(P):
        rperm[(m + HALF) % P, m] = 1.0                # out[m]=x[(m+64)%128]
    ident = np.eye(P, dtype=np.float32)
    ones_k = np.ones((P, 1), dtype=np.float32)
    ones_m = np.ones((1, P), dtype=np.float32)

    in_maps = []
    for r in range(NCORES):
        qc = slice(r * QCOLS, (r + 1) * QCOLS)
        kc = slice(NH * HD + r * HD, NH * HD + (r + 1) * HD)
        vc = slice((NH + NKV) * HD + r * HD, (NH + NKV) * HD + (r + 1) * HD)
        wqkv_s = np.ascontiguousarray(
            np.concatenate([wqkv[:, qc], wqkv[:, kc], wqkv[:, vc]], axis=1))
        wo_s = np.ascontiguousarray(wo[qc, :])
        in_maps.append({
            "hidT": hidT, "wqkv_s": wqkv_s, "wo_s": wo_s,
            "cosq": cosq, "sinq": sinq, "cosk": cosT, "sink": sinT,
            "masks": masks, "rperm": rperm, "ident": ident,
            "ones_k": ones_k, "ones_m": ones_m,
        })

    global _LAST_IN_MAPS
    _LAST_IN_MAPS = in_maps
    res = run_bass_kernel_spmd(nc, in_maps, list(range(NCORES)))
    out = res.results[0]["part"].astype(np.float64)
    for r in range(1, NCORES):
        out += res.results[r]["part"]
    return out.astype(np.float32)



# revision 4
# speedup vs baseline: 1.1450x; 1.1450x over previous
"""InternLM3 self-attention (prefill, GQA, RoPE) on 8 Trainium2 cores.

Tensor-parallel over heads: core r owns q heads 4r..4r+3 and kv head r
(wqkv column shards, wo row shards).  Each core computes its partial
output projection; the 8 partials are summed on the host.

Single fused pipeline per 512-token chunk t:
  QKV passes (one PSUM bank each) -> RoPE (q,k) -> v transpose ->
  causal attention group g=t -> softmax normalization -> wo matmul +
  bf16 store.  Emitted in one Tile scope so the scheduler overlaps
  chunk t+1's projection with chunk t's attention/output, keeping the
  PE warm (HAM at 8/8).

Everything is bf16 except q/k (f32r, softmax input precision) and
f32 PSUM/softmax internals.  Layout is fully transposed on-chip
(qkv^T = wqkv^T @ hidden^T) so scores^T = k^T-chunks @ q^T feed the
PV matmul with zero transposes; only v needs 16 tiny PE transposes.

Causal masking is a post-exp 0/1 bf16 multiply on the four diagonal
blocks; the softmax denominator accumulates on the PE into a [4, 512]
tile (one selector matmul per head) and uses the fast DVE reciprocal.
"""

import numpy as np
import ml_dtypes

import concourse.bass as bass
import concourse.bacc as bacc
import concourse.mybir as mybir
import concourse.tile as tile
from concourse.bass_utils import run_bass_kernel_spmd

T = 2048
H = 4096
NH = 32
NKV = 8
HD = 128
HALF = HD // 2
BASE = 1000000.0
NCORES = 8
QH = NH // NCORES            # 4 q heads per core
QCOLS = QH * HD              # 512
SH_COLS = QCOLS + 2 * HD     # 768 wqkv cols per core

P = 128
TC = 512                     # token chunk
NT = T // TC                 # 4
NHC = H // P                 # 32 contraction chunks
NKC = T // P                 # 16 k chunks of 128
SCALE = HD ** -0.5

f32 = mybir.dt.float32
f32r = mybir.dt.float32r
bf16 = mybir.dt.bfloat16
BF = ml_dtypes.bfloat16

_COMPILED = None
_LAST_IN_MAPS = None


def _build():
    nc = bacc.Bacc("TRN2", target_bir_lowering=False, debug=False,
                   num_devices=NCORES)

    hidT = nc.dram_tensor("hidT", [H, T], bf16, kind="ExternalInput").ap()
    wqkv_s = nc.dram_tensor("wqkv_s", [H, SH_COLS], bf16,
                            kind="ExternalInput").ap()
    wo_s = nc.dram_tensor("wo_s", [QCOLS, H], bf16,
                          kind="ExternalInput").ap()
    cosT = nc.dram_tensor("cosT", [P, T], f32, kind="ExternalInput").ap()
    sinrT = nc.dram_tensor("sinrT", [P, T], f32, kind="ExternalInput").ap()
    m01 = nc.dram_tensor("m01", [P, QH, TC], bf16,
                         kind="ExternalInput").ap()
    rperm = nc.dram_tensor("rperm", [P, P], f32r, kind="ExternalInput").ap()
    identb = nc.dram_tensor("identb", [P, P], bf16,
                            kind="ExternalInput").ap()
    selc = nc.dram_tensor("selc", [P, QH, QH], bf16,
                          kind="ExternalInput").ap()
    selr = nc.dram_tensor("selr", [QH, QH, P], bf16,
                          kind="ExternalInput").ap()
    part = nc.dram_tensor("part", [T, H], bf16, kind="ExternalOutput").ap()

    EXP = mybir.ActivationFunctionType.Exp
    MUL = mybir.AluOpType.mult
    ADD = mybir.AluOpType.add

    with tile.TileContext(nc) as tc:
        with tc.tile_pool(name="res", bufs=1) as res, \
             tc.tile_pool(name="hidp", bufs=34) as hidp, \
             tc.tile_pool(name="sb", bufs=2) as sb, \
             tc.tile_pool(name="accp", bufs=2, space="PSUM") as accp, \
             tc.tile_pool(name="stp", bufs=2, space="PSUM") as stp, \
             tc.tile_pool(name="pvp", bufs=2, space="PSUM") as pvp, \
             tc.tile_pool(name="smp", bufs=1, space="PSUM") as smp:

            # ---------------- resident SBUF ----------------
            wq = res.tile([P, NHC, SH_COLS], bf16)      # 48 KB
            wo_r = res.tile([P, QH, H], bf16)           # 32 KB
            kT = res.tile([P, T], f32r)                 # roped k^T, 8 KB
            vnat = res.tile([P, T], bf16)               # v natural, 4 KB
            ct = res.tile([P, T], f32)
            srt = res.tile([P, T], f32)
            mt = res.tile([P, QH, TC], bf16)
            rp = res.tile([P, P], f32r)
            idb = res.tile([P, P], bf16)
            slc = res.tile([P, QH, QH], bf16)
            slr = res.tile([QH, QH, P], bf16)

            # constants first on sync queue (needed ~10us in)
            nc.sync.dma_start(ct[:], cosT[:])
            nc.sync.dma_start(srt[:], sinrT[:])
            nc.sync.dma_start(mt[:], m01[:])
            nc.sync.dma_start(rp[:], rperm[:])
            nc.sync.dma_start(idb[:], identb[:])
            nc.sync.dma_start(slc[:], selc[:])
            nc.sync.dma_start(slr[:], selr[:])
            # wq by h-groups so pass c=0 starts after the first arrives
            for i in range(8):
                nc.sync.dma_start(
                    wq[:, 4 * i:4 * (i + 1), :],
                    wqkv_s[4 * i * P:4 * (i + 1) * P, :].rearrange(
                        "(h p) c -> p h c", p=P))
            # wo by head-chunks (only needed by WO, ~70us in)
            for hc in range(QH):
                nc.sync.dma_start(
                    wo_r[:, hc, :], wo_s[hc * P:(hc + 1) * P, :])

            for t in range(NT):
                tsl = slice(t * TC, (t + 1) * TC)

                # hid tiles for chunk t, split across two queues
                hts = []
                for h in range(NHC):
                    ht = hidp.tile([P, TC], bf16, tag="ht",
                                   name=f"ht_{t}_{h}")
                    eng = nc.gpsimd if h % 2 == 0 else nc.scalar
                    eng.dma_start(ht[:], hidT[h * P:(h + 1) * P, tsl])
                    hts.append(ht)

                qTg = sb.tile([P, QH, TC], f32r, tag="qTg",
                              name=f"qTg_{t}")

                # ---------- QKV projection: 6 col passes ----------
                for c in range(6):
                    qps = accp.tile([P, TC], f32, tag="acc",
                                    name=f"qps_{t}_{c}")
                    for h in range(NHC):
                        nc.tensor.matmul(
                            qps[:], wq[:, h, c * P:(c + 1) * P], hts[h][:],
                            start=(h == 0), stop=(h == NHC - 1))
                    if c < 5:
                        # RoPE: dest = x*cos + rot64(x*sinrot)
                        dest = qTg[:, c, :] if c < QH else kT[:, tsl]
                        nc.vector.tensor_tensor(dest, qps[:], ct[:, tsl],
                                                MUL)
                        bsb = sb.tile([P, TC], f32r, tag="bsb",
                                      name=f"bsb_{t}_{c}")
                        nc.vector.tensor_tensor(bsb[:], qps[:],
                                                srt[:, tsl], MUL)
                        rps = stp.tile([P, TC], f32, tag="st",
                                       name=f"rps_{t}_{c}")
                        nc.tensor.matmul(rps[:], rp[:], bsb[:],
                                         start=True, stop=True)
                        nc.vector.tensor_tensor(dest, dest.bitcast(f32),
                                                rps[:], ADD)
                    else:
                        # v: evac + 4 transposes into natural layout
                        vsb = sb.tile([P, TC], bf16, tag="vsb",
                                      name=f"vsb_{t}")
                        nc.scalar.copy(vsb[:], qps[:])
                        for j in range(4):
                            tp = smp.tile([P, P], bf16, tag="tp",
                                          name=f"tp_{t}_{j}")
                            nc.tensor.transpose(
                                tp[:], vsb[:, j * P:(j + 1) * P], idb[:])
                            kc = 4 * t + j
                            nc.vector.tensor_copy(
                                vnat[:, kc * P:(kc + 1) * P], tp[:])

                # ---------- attention group g = t ----------
                kmax = 4 * (t + 1)
                d4 = smp.tile([QH, TC], f32, tag="d4", name=f"d4_{t}")
                pvs = []
                for head in range(QH):
                    pv = pvp.tile([P, TC], f32, tag="pv",
                                  name=f"pv_{t}_{head}")
                    es = sb.tile([P, TC], bf16, tag="es", bufs=3,
                                 name=f"es_{t}_{head}")
                    eprev = None
                    for kc in range(kmax):
                        st = stp.tile([P, TC], f32, tag="st",
                                      name=f"st_{t}_{head}_{kc}")
                        nc.tensor.matmul(
                            st[:], kT[:, kc * P:(kc + 1) * P],
                            qTg[:, head, :], start=True, stop=True)
                        e = sb.tile([P, TC], bf16, tag="e", bufs=5,
                                    name=f"e_{t}_{head}_{kc}")
                        nc.scalar.activation(e[:], st[:], EXP, scale=SCALE)
                        j = kc - 4 * t
                        if j >= 0:
                            nc.vector.tensor_tensor(e[:], e[:],
                                                    mt[:, j, :], MUL)
                        if kc == 1:
                            nc.vector.tensor_tensor(es[:], eprev[:], e[:],
                                                    ADD)
                        elif kc > 1:
                            nc.vector.tensor_tensor(es[:], es[:], e[:],
                                                    ADD)
                        eprev = e
                        nc.tensor.matmul(
                            pv[:], vnat[:, kc * P:(kc + 1) * P], e[:],
                            start=(kc == 0), stop=(kc == kmax - 1))
                    pvsb = sb.tile([P, TC], bf16, tag="pvsb", bufs=5,
                                   name=f"pvsb_{t}_{head}")
                    nc.vector.tensor_copy(pvsb[:], pv[:])
                    pvs.append(pvsb)
                    # denominator: partition `head` of d4 += colsum(es)
                    nc.tensor.matmul(d4[:], slc[:, head, :], es[:],
                                     start=(head == 0),
                                     stop=(head == QH - 1))

                # softmax normalization for the whole group
                rd = sb.tile([QH, TC], f32, tag="rd", name=f"rd_{t}")
                nc.vector.reciprocal_approx_fast(rd[:], d4[:])
                rdr = sb.tile([QH, TC], bf16, tag="rdr", name=f"rdr_{t}")
                nc.vector.tensor_copy(rdr[:], rd[:])
                atg = sb.tile([P, QH, TC], bf16, tag="atg",
                              name=f"atg_{t}")
                for head in range(QH):
                    rb = stp.tile([P, TC], f32, tag="st",
                                  name=f"rb_{t}_{head}")
                    nc.tensor.matmul(rb[:], slr[:, head, :], rdr[:],
                                     start=True, stop=True)
                    rbs = sb.tile([P, TC], bf16, tag="rbs",
                                  name=f"rbs_{t}_{head}")
                    nc.scalar.copy(rbs[:], rb[:])
                    nc.vector.tensor_tensor(atg[:, head, :], pvs[head][:],
                                            rbs[:], MUL)

                # ---------- output projection for group t ----------
                for tq in range(4):
                    tcn = 4 * t + tq
                    for half in range(2):
                        osb = sb.tile([P, H // 2], bf16, tag="osb",
                                      name=f"osb_{tcn}_{half}")
                        for oi in range(4):
                            oc = half * 4 + oi
                            ops_t = accp.tile([P, TC], f32, tag="acc",
                                              name=f"o_{tcn}_{oc}")
                            for hc in range(QH):
                                nc.tensor.matmul(
                                    ops_t[:],
                                    atg[:, hc, tq * P:(tq + 1) * P],
                                    wo_r[:, hc, oc * TC:(oc + 1) * TC],
                                    start=(hc == 0), stop=(hc == QH - 1))
                            osl = osb[:, oi * TC:(oi + 1) * TC]
                            if oc % 2 == 0:
                                nc.vector.tensor_copy(osl, ops_t[:])
                            else:
                                nc.scalar.copy(osl, ops_t[:])
                        nc.sync.dma_start(
                            part[tcn * P:(tcn + 1) * P,
                                 half * (H // 2):(half + 1) * (H // 2)],
                            osb[:])

    nc.compile()
    return nc


def _tables(positions):
    pos = positions.astype(np.float64)
    inv_freq = 1.0 / (BASE ** (np.arange(HALF, dtype=np.float64) / HALF))
    freqs = pos[:, None] * inv_freq[None, :]          # [T, 64]
    cos = np.cos(freqs)
    sin = np.sin(freqs)
    cosT = np.concatenate([cos, cos], axis=1).T       # [128, T]
    sinT = np.concatenate([-sin, sin], axis=1).T      # sign folded
    sinrT = np.roll(sinT, -HALF, axis=0)              # pre-rotated by 64
    return cosT.astype(np.float32), sinrT.astype(np.float32)


def kernel(positions, hidden_states, wqkv, wo):
    global _COMPILED, _LAST_IN_MAPS
    if _COMPILED is None:
        _COMPILED = _build()
    nc = _COMPILED

    positions = np.asarray(positions)
    hidden_states = np.asarray(hidden_states)
    wqkv = np.asarray(wqkv)
    wo = np.asarray(wo)

    cosT, sinrT = _tables(positions)
    hidT = np.ascontiguousarray(hidden_states.T).astype(BF)

    # 0/1 causal masks for the 4 diagonal sub-blocks, ^T layout [k, q]
    kl = np.arange(P)[:, None]
    ql = np.arange(TC)[None, :]
    m01 = np.stack(
        [np.where(P * j + kl <= ql, 1.0, 0.0) for j in range(4)],
        axis=1).astype(BF)                            # [128, 4, 512]

    rperm = np.zeros((P, P), dtype=np.float32)
    for m in range(P):
        rperm[(m + HALF) % P, m] = 1.0                # out[m]=x[(m+64)%128]
    identb = np.eye(P, dtype=BF)
    selc = np.zeros((P, QH, QH), dtype=BF)
    selr = np.zeros((QH, QH, P), dtype=BF)
    for h in range(QH):
        selc[:, h, h] = 1.0
        selr[h, h, :] = 1.0

    in_maps = []
    for r in range(NCORES):
        qc = slice(r * QCOLS, (r + 1) * QCOLS)
        kc = slice(NH * HD + r * HD, NH * HD + (r + 1) * HD)
        vc = slice((NH + NKV) * HD + r * HD, (NH + NKV) * HD + (r + 1) * HD)
        wqkv_s = np.ascontiguousarray(
            np.concatenate([wqkv[:, qc], wqkv[:, kc], wqkv[:, vc]],
                           axis=1)).astype(BF)
        wo_s = np.ascontiguousarray(wo[qc, :]).astype(BF)
        in_maps.append({
            "hidT": hidT, "wqkv_s": wqkv_s, "wo_s": wo_s,
            "cosT": cosT, "sinrT": sinrT, "m01": m01, "rperm": rperm,
            "identb": identb, "selc": selc, "selr": selr,
        })

    _LAST_IN_MAPS = in_maps
    res = run_bass_kernel_spmd(nc, in_maps, list(range(NCORES)))
    out = res.results[0]["part"].astype(np.float64)
    for r in range(1, NCORES):
        out += res.results[r]["part"].astype(np.float64)
    return out.astype(np.float32)


# revision 12
# speedup vs baseline: 1.3832x; 1.2080x over previous
"""InternLM3 self-attention (prefill, GQA, RoPE) on 8 Trainium2 cores.

Tensor-parallel over heads: core r owns q heads 4r..4r+3 and kv head r
(wqkv column shards, wo row shards).  Each core computes its partial
output projection; the 8 partials are summed on the host.

Single fused pipeline per 512-token chunk t:
  QKV passes (one PSUM bank each) -> RoPE (q,k) -> v transpose ->
  causal attention group g=t -> softmax normalization -> wo matmul +
  bf16 store.  Emitted in one Tile scope so the scheduler overlaps
  chunk t+1's projection with chunk t's attention/output, keeping the
  PE warm (HAM at 8/8).

Everything is bf16 except q/k (f32r, softmax input precision) and
f32 PSUM/softmax internals.  Layout is fully transposed on-chip
(qkv^T = wqkv^T @ hidden^T) so scores^T = k^T-chunks @ q^T feed the
PV matmul with zero transposes; only v needs 16 tiny PE transposes.

Causal masking is a post-exp 0/1 bf16 multiply on the four diagonal
blocks; the softmax denominator accumulates on the PE into a [4, 512]
tile (one selector matmul per head) and uses the fast DVE reciprocal.
"""

import numpy as np
import ml_dtypes

import concourse.bass as bass
import concourse.bacc as bacc
import concourse.mybir as mybir
import concourse.tile as tile
from concourse.bass_utils import run_bass_kernel_spmd

T = 2048
H = 4096
NH = 32
NKV = 8
HD = 128
HALF = HD // 2
BASE = 1000000.0
NCORES = 8
QH = NH // NCORES            # 4 q heads per core
QCOLS = QH * HD              # 512
SH_COLS = QCOLS + 2 * HD     # 768 wqkv cols per core

P = 128
TC = 512                     # token chunk
NT = T // TC                 # 4
NHC = H // P                 # 32 contraction chunks
NKC = T // P                 # 16 k chunks of 128
SCALE = HD ** -0.5

f32 = mybir.dt.float32
f32r = mybir.dt.float32r
bf16 = mybir.dt.bfloat16
BF = ml_dtypes.bfloat16

_COMPILED = None
_LAST_IN_MAPS = None


def _build():
    nc = bacc.Bacc("TRN2", target_bir_lowering=False, debug=False,
                   num_devices=NCORES)

    hidT = nc.dram_tensor("hidT", [H, T], bf16, kind="ExternalInput").ap()
    # wqkv shard pre-transposed host-side: [c-chunk, p, h, col] so one
    # column-chunk = one DMA with 8 KB contiguous per partition
    wqd = nc.dram_tensor("wqd", [6, P, NHC, P], bf16,
                         kind="ExternalInput").ap()
    wo_s = nc.dram_tensor("wo_s", [QCOLS, H], bf16,
                          kind="ExternalInput").ap()
    cosT = nc.dram_tensor("cosT", [P, T], f32, kind="ExternalInput").ap()
    sinrT = nc.dram_tensor("sinrT", [P, T], f32, kind="ExternalInput").ap()
    m01 = nc.dram_tensor("m01", [P, QH, TC], bf16,
                         kind="ExternalInput").ap()
    rperm = nc.dram_tensor("rperm", [P, P], bf16, kind="ExternalInput").ap()
    identb = nc.dram_tensor("identb", [P, P], bf16,
                            kind="ExternalInput").ap()
    selc = nc.dram_tensor("selc", [P, QH, QH], bf16,
                          kind="ExternalInput").ap()
    selr = nc.dram_tensor("selr", [QH, QH, P], bf16,
                          kind="ExternalInput").ap()
    part = nc.dram_tensor("part", [T, H], bf16, kind="ExternalOutput").ap()

    EXP = mybir.ActivationFunctionType.Exp
    MUL = mybir.AluOpType.mult
    ADD = mybir.AluOpType.add

    with tile.TileContext(nc) as tc:
        with tc.tile_pool(name="res", bufs=1) as res, \
             tc.tile_pool(name="hidp", bufs=36) as hidp, \
             tc.tile_pool(name="sb", bufs=2) as sb, \
             tc.tile_pool(name="accp", bufs=2, space="PSUM") as accp, \
             tc.tile_pool(name="stp", bufs=3, space="PSUM") as stp, \
             tc.tile_pool(name="pvp", bufs=2, space="PSUM") as pvp, \
             tc.tile_pool(name="smp", bufs=1, space="PSUM") as smp:

            # ---------------- resident SBUF ----------------
            wq = res.tile([P, 6, NHC, P], bf16)         # 48 KB
            wo_r = res.tile([P, QH, H], bf16)           # 32 KB
            kT = res.tile([P, T], bf16)                 # roped k^T, 4 KB
            vnat = res.tile([P, T], bf16)               # v natural, 4 KB
            ct = res.tile([P, T], f32)
            srt = res.tile([P, T], f32)
            mt = res.tile([P, QH, TC], bf16)
            rp = res.tile([P, P], bf16)
            idb = res.tile([P, P], bf16)
            slc = res.tile([P, QH, QH], bf16)
            slr = res.tile([QH, QH, P], bf16)

            # constants first on sync queue (needed ~10us in)
            nc.sync.dma_start(ct[:], cosT[:])
            nc.sync.dma_start(srt[:], sinrT[:])
            nc.sync.dma_start(mt[:], m01[:])
            nc.sync.dma_start(rp[:], rperm[:])
            nc.sync.dma_start(idb[:], identb[:])
            nc.sync.dma_start(slc[:], selc[:])
            nc.sync.dma_start(slr[:], selr[:])
            # wq by column-chunks (contiguous per partition), 3 queues,
            # so pass c starts as soon as its chunk lands
            for c in range(6):
                eng = (nc.sync, nc.scalar, nc.gpsimd)[c % 3]
                eng.dma_start(wq[:, c, :, :], wqd[c, :, :, :])
            # wo by head-chunks (only needed by WO, ~70us in)
            for hc in range(QH):
                nc.sync.dma_start(
                    wo_r[:, hc, :], wo_s[hc * P:(hc + 1) * P, :])

            for t in range(NT):
                tsl = slice(t * TC, (t + 1) * TC)

                # hid tiles for chunk t, split across two queues
                hts = []
                for h in range(NHC):
                    ht = hidp.tile([P, TC], bf16, tag="ht",
                                   name=f"ht_{t}_{h}")
                    eng = nc.gpsimd if h % 2 == 0 else nc.scalar
                    eng.dma_start(ht[:], hidT[h * P:(h + 1) * P, tsl])
                    hts.append(ht)

                qTg = sb.tile([P, QH, TC], bf16, tag="qTg",
                              name=f"qTg_{t}")

                # ---------- QKV projection: 6 col passes ----------
                for c in range(6):
                    qps = accp.tile([P, TC], f32, tag="acc",
                                    name=f"qps_{t}_{c}")
                    for h in range(NHC):
                        nc.tensor.matmul(
                            qps[:], wq[:, c, h, :], hts[h][:],
                            start=(h == 0), stop=(h == NHC - 1))
                    if c < 5:
                        # RoPE: dest = x*cos + rot64(x*sinrot)
                        dest = qTg[:, c, :] if c < QH else kT[:, tsl]
                        acos = sb.tile([P, TC], f32, tag="acos",
                                       name=f"acos_{t}_{c}")
                        nc.vector.tensor_tensor(acos[:], qps[:],
                                                ct[:, tsl], MUL)
                        bsb = sb.tile([P, TC], bf16, tag="bsb",
                                      name=f"bsb_{t}_{c}")
                        nc.vector.tensor_tensor(bsb[:], qps[:],
                                                srt[:, tsl], MUL)
                        rps = stp.tile([P, TC], f32, tag="st",
                                       name=f"rps_{t}_{c}")
                        nc.tensor.matmul(rps[:], rp[:], bsb[:],
                                         start=True, stop=True)
                        nc.vector.tensor_tensor(dest, acos[:], rps[:],
                                                ADD)
                    else:
                        # v: evac + 4 transposes into natural layout
                        vsb = sb.tile([P, TC], bf16, tag="vsb",
                                      name=f"vsb_{t}")
                        nc.scalar.copy(vsb[:], qps[:])
                        for j in range(4):
                            tp = stp.tile([P, P], bf16, tag="st",
                                          name=f"tp_{t}_{j}")
                            nc.tensor.transpose(
                                tp[:], vsb[:, j * P:(j + 1) * P], idb[:])
                            kc = 4 * t + j
                            nc.vector.tensor_copy(
                                vnat[:, kc * P:(kc + 1) * P], tp[:])

                # ---------- attention group g = t ----------
                kmax = 4 * (t + 1)
                d4 = smp.tile([QH, TC], f32, tag="d4", name=f"d4_{t}")
                pvs = []
                for head in range(QH):
                    pv = pvp.tile([P, TC], f32, tag="pv",
                                  name=f"pv_{t}_{head}")
                    es = sb.tile([P, TC], bf16, tag="es", bufs=3,
                                 name=f"es_{t}_{head}")
                    eprev = None
                    for kc in range(kmax):
                        st = stp.tile([P, TC], f32, tag="st",
                                      name=f"st_{t}_{head}_{kc}")
                        nc.tensor.matmul(
                            st[:], kT[:, kc * P:(kc + 1) * P],
                            qTg[:, head, :], start=True, stop=True)
                        e = sb.tile([P, TC], bf16, tag="e", bufs=6,
                                    name=f"e_{t}_{head}_{kc}")
                        nc.scalar.activation(e[:], st[:], EXP, scale=SCALE)
                        j = kc - 4 * t
                        if j >= 0:
                            nc.vector.tensor_tensor(e[:], e[:],
                                                    mt[:, j, :], MUL)
                        if kc == 1:
                            nc.vector.tensor_tensor(es[:], eprev[:], e[:],
                                                    ADD)
                        elif kc > 1:
                            nc.vector.tensor_tensor(es[:], es[:], e[:],
                                                    ADD)
                        eprev = e
                        nc.tensor.matmul(
                            pv[:], vnat[:, kc * P:(kc + 1) * P], e[:],
                            start=(kc == 0), stop=(kc == kmax - 1))
                    pvsb = sb.tile([P, TC], bf16, tag="pvsb", bufs=5,
                                   name=f"pvsb_{t}_{head}")
                    nc.vector.tensor_copy(pvsb[:], pv[:])
                    pvs.append(pvsb)
                    # denominator: partition `head` of d4 += colsum(es)
                    nc.tensor.matmul(d4[:], slc[:, head, :], es[:],
                                     start=(head == 0),
                                     stop=(head == QH - 1))

                # softmax normalization for the whole group
                rd = sb.tile([QH, TC], f32, tag="rd", name=f"rd_{t}")
                nc.vector.reciprocal_approx_fast(rd[:], d4[:])
                rdr = sb.tile([QH, TC], bf16, tag="rdr", name=f"rdr_{t}")
                nc.vector.tensor_copy(rdr[:], rd[:])
                atg = sb.tile([P, QH, TC], bf16, tag="atg",
                              name=f"atg_{t}")
                for head in range(QH):
                    rb = stp.tile([P, TC], f32, tag="st",
                                  name=f"rb_{t}_{head}")
                    nc.tensor.matmul(rb[:], slr[:, head, :], rdr[:],
                                     start=True, stop=True)
                    rbs = sb.tile([P, TC], bf16, tag="rbs",
                                  name=f"rbs_{t}_{head}")
                    nc.scalar.copy(rbs[:], rb[:])
                    nc.vector.tensor_tensor(atg[:, head, :], pvs[head][:],
                                            rbs[:], MUL)

                # ---------- output projection for group t ----------
                for tq in range(4):
                    tcn = 4 * t + tq
                    for half in range(2):
                        osb = sb.tile([P, H // 2], bf16, tag="osb",
                                      name=f"osb_{tcn}_{half}")
                        for oi in range(4):
                            oc = half * 4 + oi
                            ops_t = accp.tile([P, TC], f32, tag="acc",
                                              name=f"o_{tcn}_{oc}")
                            for hc in range(QH):
                                nc.tensor.matmul(
                                    ops_t[:],
                                    atg[:, hc, tq * P:(tq + 1) * P],
                                    wo_r[:, hc, oc * TC:(oc + 1) * TC],
                                    start=(hc == 0), stop=(hc == QH - 1))
                            osl = osb[:, oi * TC:(oi + 1) * TC]
                            if oc % 2 == 0:
                                nc.vector.tensor_copy(osl, ops_t[:])
                            else:
                                nc.scalar.copy(osl, ops_t[:])
                        nc.sync.dma_start(
                            part[tcn * P:(tcn + 1) * P,
                                 half * (H // 2):(half + 1) * (H // 2)],
                            osb[:])

    nc.compile()
    return nc


def _tables(positions):
    pos = positions.astype(np.float64)
    inv_freq = 1.0 / (BASE ** (np.arange(HALF, dtype=np.float64) / HALF))
    freqs = pos[:, None] * inv_freq[None, :]          # [T, 64]
    cos = np.cos(freqs)
    sin = np.sin(freqs)
    cosT = np.concatenate([cos, cos], axis=1).T       # [128, T]
    sinT = np.concatenate([-sin, sin], axis=1).T      # sign folded
    sinrT = np.roll(sinT, -HALF, axis=0)              # pre-rotated by 64
    return cosT.astype(np.float32), sinrT.astype(np.float32)


def kernel(positions, hidden_states, wqkv, wo):
    global _COMPILED, _LAST_IN_MAPS
    if _COMPILED is None:
        _COMPILED = _build()
    nc = _COMPILED

    positions = np.asarray(positions)
    hidden_states = np.asarray(hidden_states)
    wqkv = np.asarray(wqkv)
    wo = np.asarray(wo)

    cosT, sinrT = _tables(positions)
    hidT = np.ascontiguousarray(hidden_states.T).astype(BF)

    # 0/1 causal masks for the 4 diagonal sub-blocks, ^T layout [k, q]
    kl = np.arange(P)[:, None]
    ql = np.arange(TC)[None, :]
    m01 = np.stack(
        [np.where(P * j + kl <= ql, 1.0, 0.0) for j in range(4)],
        axis=1).astype(BF)                            # [128, 4, 512]

    rperm = np.zeros((P, P), dtype=np.float32)
    for m in range(P):
        rperm[(m + HALF) % P, m] = 1.0                # out[m]=x[(m+64)%128]
    rperm = rperm.astype(BF)
    identb = np.eye(P, dtype=BF)
    selc = np.zeros((P, QH, QH), dtype=BF)
    selr = np.zeros((QH, QH, P), dtype=BF)
    for h in range(QH):
        selc[:, h, h] = 1.0
        selr[h, h, :] = 1.0

    in_maps = []
    for r in range(NCORES):
        qc = slice(r * QCOLS, (r + 1) * QCOLS)
        kc = slice(NH * HD + r * HD, NH * HD + (r + 1) * HD)
        vc = slice((NH + NKV) * HD + r * HD, (NH + NKV) * HD + (r + 1) * HD)
        wqkv_s = np.concatenate([wqkv[:, qc], wqkv[:, kc], wqkv[:, vc]],
                                axis=1)
        # [c-chunk, p, h, col] so each c-chunk is contiguous per partition
        wqd = np.ascontiguousarray(
            wqkv_s.reshape(NHC, P, 6, P).transpose(2, 1, 0, 3)).astype(BF)
        wo_s = np.ascontiguousarray(wo[qc, :]).astype(BF)
        in_maps.append({
            "hidT": hidT, "wqd": wqd, "wo_s": wo_s,
            "cosT": cosT, "sinrT": sinrT, "m01": m01, "rperm": rperm,
            "identb": identb, "selc": selc, "selr": selr,
        })

    _LAST_IN_MAPS = in_maps
    res = run_bass_kernel_spmd(nc, in_maps, list(range(NCORES)))
    out = res.results[0]["part"].astype(np.float64)
    for r in range(1, NCORES):
        out += res.results[r]["part"].astype(np.float64)
    return out.astype(np.float32)


# revision 17
# speedup vs baseline: 1.4970x; 1.0823x over previous
"""InternLM3 self-attention (prefill, GQA, RoPE) on 8 Trainium2 cores.

Tensor-parallel over heads: core r owns q heads 4r..4r+3 and kv head r
(wqkv column shards, wo row shards).  Each core computes its partial
output projection; the 8 partials are summed on the host.

Single fused pipeline per 512-token chunk t:
  QKV passes (one PSUM bank each) -> RoPE (q,k) -> v transpose ->
  causal attention group g=t -> softmax normalization -> wo matmul +
  bf16 store.  Emitted in one Tile scope so the scheduler overlaps
  chunk t+1's projection with chunk t's attention/output, keeping the
  PE warm (HAM at 8/8).

Everything is bf16 except q/k (f32r, softmax input precision) and
f32 PSUM/softmax internals.  Layout is fully transposed on-chip
(qkv^T = wqkv^T @ hidden^T) so scores^T = k^T-chunks @ q^T feed the
PV matmul with zero transposes; only v needs 16 tiny PE transposes.

Causal masking is a post-exp 0/1 bf16 multiply on the four diagonal
blocks; the softmax denominator accumulates on the PE into a [4, 512]
tile (one selector matmul per head) and uses the fast DVE reciprocal.
"""

import numpy as np
import ml_dtypes

import concourse.bass as bass
import concourse.bacc as bacc
import concourse.mybir as mybir
import concourse.tile as tile
from concourse.bass_utils import run_bass_kernel_spmd

T = 2048
H = 4096
NH = 32
NKV = 8
HD = 128
HALF = HD // 2
BASE = 1000000.0
NCORES = 8
QH = NH // NCORES            # 4 q heads per core
QCOLS = QH * HD              # 512
SH_COLS = QCOLS + 2 * HD     # 768 wqkv cols per core

P = 128
TC = 512                     # token chunk
NT = T // TC                 # 4
NHC = H // P                 # 32 contraction chunks
NKC = T // P                 # 16 k chunks of 128
SCALE = HD ** -0.5

f32 = mybir.dt.float32
f32r = mybir.dt.float32r
bf16 = mybir.dt.bfloat16
BF = ml_dtypes.bfloat16

_COMPILED = None
_LAST_IN_MAPS = None


def _build():
    nc = bacc.Bacc("TRN2", target_bir_lowering=False, debug=False,
                   num_devices=NCORES)

    hidT = nc.dram_tensor("hidT", [H, T], bf16, kind="ExternalInput").ap()
    # wqkv shard pre-transposed host-side: [c-chunk, p, h, col] so one
    # column-chunk = one DMA with 8 KB contiguous per partition
    wqd = nc.dram_tensor("wqd", [6, P, NHC, P], bf16,
                         kind="ExternalInput").ap()
    wo_s = nc.dram_tensor("wo_s", [QCOLS, H], bf16,
                          kind="ExternalInput").ap()
    cosT = nc.dram_tensor("cosT", [P, T], f32, kind="ExternalInput").ap()
    sinrT = nc.dram_tensor("sinrT", [P, T], f32, kind="ExternalInput").ap()
    m01 = nc.dram_tensor("m01", [P, QH, TC], bf16,
                         kind="ExternalInput").ap()
    rperm = nc.dram_tensor("rperm", [P, P], bf16, kind="ExternalInput").ap()
    identb = nc.dram_tensor("identb", [P, P], bf16,
                            kind="ExternalInput").ap()
    selc = nc.dram_tensor("selc", [P, QH, QH], bf16,
                          kind="ExternalInput").ap()
    selr = nc.dram_tensor("selr", [QH, QH, P], bf16,
                          kind="ExternalInput").ap()
    part = nc.dram_tensor("part", [T, H], bf16, kind="ExternalOutput").ap()

    EXP = mybir.ActivationFunctionType.Exp
    MUL = mybir.AluOpType.mult
    ADD = mybir.AluOpType.add

    with tile.TileContext(nc) as tc:
        with tc.tile_pool(name="res", bufs=1) as res, \
             tc.tile_pool(name="hidp", bufs=10) as hidp, \
             tc.tile_pool(name="sb", bufs=2) as sb, \
             tc.tile_pool(name="accp", bufs=2, space="PSUM") as accp, \
             tc.tile_pool(name="stp", bufs=3, space="PSUM") as stp, \
             tc.tile_pool(name="pvp", bufs=2, space="PSUM") as pvp, \
             tc.tile_pool(name="smp", bufs=1, space="PSUM") as smp:

            # ---------------- resident SBUF ----------------
            wq = res.tile([P, 6, NHC, P], bf16)         # 48 KB
            wo_r = res.tile([P, QH, H], bf16)           # 32 KB
            kT = res.tile([P, T], bf16)                 # roped k^T, 4 KB
            vnat = res.tile([P, T], bf16)               # v natural, 4 KB
            ct = res.tile([P, T], f32)
            srt = res.tile([P, T], f32)
            mt = res.tile([P, QH, TC], bf16)
            rp = res.tile([P, P], bf16)
            idb = res.tile([P, P], bf16)
            slc = res.tile([P, QH, QH], bf16)
            slr = res.tile([QH, QH, P], bf16)

            # startup order tuned so pass c=0 (wq chunk 0 + all hid
            # groups of chunk 0) is fed within ~15us across 3 queues
            QS, QA, QG = nc.sync, nc.scalar, nc.gpsimd

            def load_hid_group(t, g, eng):
                # one DMA: 4 h-chunks of hidden for token chunk t
                ht = hidp.tile([P, 4, TC], bf16, tag="ht",
                               name=f"ht_{t}_{g}")
                eng.dma_start(
                    ht[:], hidT[4 * g * P:4 * (g + 1) * P,
                                t * TC:(t + 1) * TC].rearrange(
                                    "(h p) n -> p h n", p=P))
                return ht

            nc.sync.dma_start(wq[:, 0, :, :], wqd[0, :, :, :])
            hts0 = [None] * 8
            for g, eng in ((0, QS), (1, QA), (2, QG), (3, QS), (4, QA),
                           (5, QG), (6, QA), (7, QG)):
                hts0[g] = load_hid_group(0, g, eng)
            nc.scalar.dma_start(wq[:, 1, :, :], wqd[1, :, :, :])
            nc.gpsimd.dma_start(wq[:, 2, :, :], wqd[2, :, :, :])
            nc.sync.dma_start(ct[:], cosT[:])
            nc.sync.dma_start(srt[:], sinrT[:])
            nc.scalar.dma_start(wq[:, 3, :, :], wqd[3, :, :, :])
            nc.gpsimd.dma_start(wq[:, 4, :, :], wqd[4, :, :, :])
            nc.scalar.dma_start(wq[:, 5, :, :], wqd[5, :, :, :])
            nc.sync.dma_start(mt[:], m01[:])
            nc.sync.dma_start(rp[:], rperm[:])
            nc.sync.dma_start(idb[:], identb[:])
            nc.sync.dma_start(slc[:], selc[:])
            nc.sync.dma_start(slr[:], selr[:])
            # wo by head-chunks (only needed by WO, ~70us in)
            for hc in range(QH):
                nc.sync.dma_start(
                    wo_r[:, hc, :], wo_s[hc * P:(hc + 1) * P, :])

            for t in range(NT):
                tsl = slice(t * TC, (t + 1) * TC)

                # hid group tiles for chunk t (8 DMAs over 3 queues)
                if t == 0:
                    htg = hts0
                else:
                    htg = [load_hid_group(t, g, (QS, QA, QG)[g % 3])
                           for g in range(8)]
                hts = [htg[h // 4][:, h % 4, :] for h in range(NHC)]

                qTg = sb.tile([P, QH, TC], bf16, tag="qTg",
                              name=f"qTg_{t}")

                # ---------- QKV projection: 6 col passes ----------
                for c in range(6):
                    qps = accp.tile([P, TC], f32, tag="acc",
                                    name=f"qps_{t}_{c}")
                    for h in range(NHC):
                        nc.tensor.matmul(
                            qps[:], wq[:, c, h, :], hts[h],
                            start=(h == 0), stop=(h == NHC - 1))
                    if c < 5:
                        # RoPE: dest = x*cos + rot64(x*sinrot)
                        dest = qTg[:, c, :] if c < QH else kT[:, tsl]
                        acos = sb.tile([P, TC], f32, tag="acos",
                                       name=f"acos_{t}_{c}")
                        nc.vector.tensor_tensor(acos[:], qps[:],
                                                ct[:, tsl], MUL)
                        bsb = sb.tile([P, TC], bf16, tag="bsb",
                                      name=f"bsb_{t}_{c}")
                        nc.vector.tensor_tensor(bsb[:], qps[:],
                                                srt[:, tsl], MUL)
                        rps = stp.tile([P, TC], f32, tag="st",
                                       name=f"rps_{t}_{c}")
                        nc.tensor.matmul(rps[:], rp[:], bsb[:],
                                         start=True, stop=True)
                        nc.vector.tensor_tensor(dest, acos[:], rps[:],
                                                ADD)
                    else:
                        # v: evac + 4 transposes into natural layout
                        vsb = sb.tile([P, TC], bf16, tag="vsb",
                                      name=f"vsb_{t}")
                        nc.vector.tensor_copy(vsb[:], qps[:])
                        for j in range(4):
                            tp = stp.tile([P, P], bf16, tag="st",
                                          name=f"tp_{t}_{j}")
                            nc.tensor.transpose(
                                tp[:], vsb[:, j * P:(j + 1) * P], idb[:])
                            kc = 4 * t + j
                            nc.vector.tensor_copy(
                                vnat[:, kc * P:(kc + 1) * P], tp[:])

                # ---------- attention group g = t ----------
                kmax = 4 * (t + 1)
                d4 = smp.tile([QH, TC], f32, tag="d4", name=f"d4_{t}")
                pvs = []
                for head in range(QH):
                    pv = pvp.tile([P, TC], f32, tag="pv",
                                  name=f"pv_{t}_{head}")
                    es = sb.tile([P, TC], bf16, tag="es", bufs=3,
                                 name=f"es_{t}_{head}")
                    for kc in range(kmax):
                        # diagonal block j: only q >= 128*j is unmasked,
                        # so shrink the moving operand to N = 512-128*j
                        j = kc - 4 * t
                        off = max(0, j) * P
                        n = TC - off
                        qsl = slice(off, TC)
                        st = stp.tile([P, TC], f32, tag="st",
                                      name=f"st_{t}_{head}_{kc}")
                        nc.tensor.matmul(
                            st[:, :n], kT[:, kc * P:(kc + 1) * P],
                            qTg[:, head, qsl], start=True, stop=True)
                        e = sb.tile([P, TC], bf16, tag="e", bufs=6,
                                    name=f"e_{t}_{head}_{kc}")
                        nc.scalar.activation(e[:, :n], st[:, :n], EXP,
                                             scale=SCALE)
                        if j >= 0:
                            nc.vector.tensor_tensor(e[:, :n], e[:, :n],
                                                    mt[:, j, qsl], MUL)
                        if kc == 0:
                            nc.vector.tensor_copy(es[:], e[:])
                        else:
                            nc.vector.tensor_tensor(es[:, qsl],
                                                    es[:, qsl], e[:, :n],
                                                    ADD)
                        nc.tensor.matmul(
                            pv[:, qsl], vnat[:, kc * P:(kc + 1) * P],
                            e[:, :n], start=(kc == 0),
                            stop=(kc == kmax - 1))
                    pvsb = sb.tile([P, TC], bf16, tag="pvsb", bufs=5,
                                   name=f"pvsb_{t}_{head}")
                    nc.vector.tensor_copy(pvsb[:], pv[:])
                    pvs.append(pvsb)
                    # denominator: partition `head` of d4 += colsum(es)
                    nc.tensor.matmul(d4[:], slc[:, head, :], es[:],
                                     start=(head == 0),
                                     stop=(head == QH - 1))

                # softmax normalization for the whole group
                rd = sb.tile([QH, TC], f32, tag="rd", name=f"rd_{t}")
                nc.vector.reciprocal_approx_fast(rd[:], d4[:])
                rdr = sb.tile([QH, TC], bf16, tag="rdr", name=f"rdr_{t}")
                nc.vector.tensor_copy(rdr[:], rd[:])
                atg = sb.tile([P, QH, TC], bf16, tag="atg",
                              name=f"atg_{t}")
                for head in range(QH):
                    rb = stp.tile([P, TC], f32, tag="st",
                                  name=f"rb_{t}_{head}")
                    nc.tensor.matmul(rb[:], slr[:, head, :], rdr[:],
                                     start=True, stop=True)
                    rbs = sb.tile([P, TC], bf16, tag="rbs",
                                  name=f"rbs_{t}_{head}")
                    nc.scalar.copy(rbs[:], rb[:])
                    nc.vector.tensor_tensor(atg[:, head, :], pvs[head][:],
                                            rbs[:], MUL)

                # ---------- output projection for group t ----------
                for tq in range(4):
                    tcn = 4 * t + tq
                    for half in range(2):
                        osb = sb.tile([P, H // 2], bf16, tag="osb",
                                      name=f"osb_{tcn}_{half}")
                        for oi in range(4):
                            oc = half * 4 + oi
                            ops_t = accp.tile([P, TC], f32, tag="acc",
                                              name=f"o_{tcn}_{oc}")
                            for hc in range(QH):
                                nc.tensor.matmul(
                                    ops_t[:],
                                    atg[:, hc, tq * P:(tq + 1) * P],
                                    wo_r[:, hc, oc * TC:(oc + 1) * TC],
                                    start=(hc == 0), stop=(hc == QH - 1))
                            osl = osb[:, oi * TC:(oi + 1) * TC]
                            if oc % 2 == 0:
                                nc.vector.tensor_copy(osl, ops_t[:])
                            else:
                                nc.scalar.copy(osl, ops_t[:])
                        nc.sync.dma_start(
                            part[tcn * P:(tcn + 1) * P,
                                 half * (H // 2):(half + 1) * (H // 2)],
                            osb[:])

    nc.compile()
    return nc


def _tables(positions):
    pos = positions.astype(np.float64)
    inv_freq = 1.0 / (BASE ** (np.arange(HALF, dtype=np.float64) / HALF))
    freqs = pos[:, None] * inv_freq[None, :]          # [T, 64]
    cos = np.cos(freqs)
    sin = np.sin(freqs)
    cosT = np.concatenate([cos, cos], axis=1).T       # [128, T]
    sinT = np.concatenate([-sin, sin], axis=1).T      # sign folded
    sinrT = np.roll(sinT, -HALF, axis=0)              # pre-rotated by 64
    return cosT.astype(np.float32), sinrT.astype(np.float32)


def kernel(positions, hidden_states, wqkv, wo):
    global _COMPILED, _LAST_IN_MAPS
    if _COMPILED is None:
        _COMPILED = _build()
    nc = _COMPILED

    positions = np.asarray(positions)
    hidden_states = np.asarray(hidden_states)
    wqkv = np.asarray(wqkv)
    wo = np.asarray(wo)

    cosT, sinrT = _tables(positions)
    hidT = np.ascontiguousarray(hidden_states.T).astype(BF)

    # 0/1 causal masks for the 4 diagonal sub-blocks, ^T layout [k, q]
    kl = np.arange(P)[:, None]
    ql = np.arange(TC)[None, :]
    m01 = np.stack(
        [np.where(P * j + kl <= ql, 1.0, 0.0) for j in range(4)],
        axis=1).astype(BF)                            # [128, 4, 512]

    rperm = np.zeros((P, P), dtype=np.float32)
    for m in range(P):
        rperm[(m + HALF) % P, m] = 1.0                # out[m]=x[(m+64)%128]
    rperm = rperm.astype(BF)
    identb = np.eye(P, dtype=BF)
    selc = np.zeros((P, QH, QH), dtype=BF)
    selr = np.zeros((QH, QH, P), dtype=BF)
    for h in range(QH):
        selc[:, h, h] = 1.0
        selr[h, h, :] = 1.0

    in_maps = []
    for r in range(NCORES):
        qc = slice(r * QCOLS, (r + 1) * QCOLS)
        kc = slice(NH * HD + r * HD, NH * HD + (r + 1) * HD)
        vc = slice((NH + NKV) * HD + r * HD, (NH + NKV) * HD + (r + 1) * HD)
        wqkv_s = np.concatenate([wqkv[:, qc], wqkv[:, kc], wqkv[:, vc]],
                                axis=1)
        # [c-chunk, p, h, col] so each c-chunk is contiguous per partition
        wqd = np.ascontiguousarray(
            wqkv_s.reshape(NHC, P, 6, P).transpose(2, 1, 0, 3)).astype(BF)
        wo_s = np.ascontiguousarray(wo[qc, :]).astype(BF)
        in_maps.append({
            "hidT": hidT, "wqd": wqd, "wo_s": wo_s,
            "cosT": cosT, "sinrT": sinrT, "m01": m01, "rperm": rperm,
            "identb": identb, "selc": selc, "selr": selr,
        })

    _LAST_IN_MAPS = in_maps
    res = run_bass_kernel_spmd(nc, in_maps, list(range(NCORES)))
    out = res.results[0]["part"].astype(np.float64)
    for r in range(1, NCORES):
        out += res.results[r]["part"].astype(np.float64)
    return out.astype(np.float32)
